# revision 4
# baseline (speedup 1.0000x reference)
"""Trainium2 Bass kernel for nn_AttentionS2 (spherical self-attention), v2.

Module: y = p_w @ softmax_k(q k^T / sqrt(hd) + log_quad_w[k]) v + p_b
with q/k/v = 1x1-conv projections of the same input (self-attention),
B=1, C=512, H=W=64 (4096 tokens), 8 heads, head_dim=64.

Sharding: one head per NeuronCore (8 cores). Per core:
  1. paired q+k projection (both heads' 64-channel blocks in one 128-wide
     PSUM tile), v^T token-major tiles; all matmul operands bf16 (host
     pre-casts inputs; FWL makes bf16 weight loads ~2x faster, DMA halves)
  2. S^T = k^T q in (key x query) orientation, 128-key x 1024-query tiles
  3. exp(scale*S^T + log_qw[key]) on ACT (per-partition bias = per-key),
     bf16 output; ACT is the rate limiter (~128us busy/core), the pipeline
     is built to keep it saturated
  4. [v^T | 1]^T @ P accumulates numerator rows 0..63 + denominator row 64
     in PSUM; reciprocal + partition-broadcast (K=1 matmul) normalize
  5. token-striped AllToAll: core c owns token tiles {8q+c : q in 0..3};
     after each 1024-query chunk q one A2A piece fires, so 3 of 4 pieces
     overlap the remaining attention compute
  6. full output projection p_w on the core's 4x128 token tiles
Host casts inputs to bf16, slices weights per head, restripes the output.

Accumulation stays fp32 in PSUM; softmax skips max-subtraction (logits
are q.k/8 + log(quad weights), bounded well inside fp32 exp range).
"""

import contextlib
import sys
import types

import numpy as np
import ml_dtypes

import concourse.bass as bass
import concourse.bacc as bacc
import concourse.tile as tile
from concourse import mybir
from concourse import bass_utils

# This container has no axon NTFF profile hook; shim the module so
# run_bass_kernel_spmd(trace=True) degrades gracefully instead of raising.
try:  # pragma: no cover
    import antenv.axon_hooks  # noqa: F401
except Exception:  # ModuleNotFoundError, or antenv missing entirely
    try:
        import antenv  # noqa: F401
    except Exception:
        antenv_mod = types.ModuleType("antenv")
        sys.modules["antenv"] = antenv_mod
    shim = types.ModuleType("antenv.axon_hooks")
    shim.get_axon_ntff_profile_hook = lambda: None
    sys.modules["antenv.axon_hooks"] = shim

F32 = mybir.dt.float32
F32R = mybir.dt.float32r
BF16 = mybir.dt.bfloat16
AF = mybir.ActivationFunctionType
NPBF = ml_dtypes.bfloat16

C = 512          # channels
T = 4096         # tokens (H*W)
HD = 64          # head dim
NCORES = 8
NKT = T // 128   # 32 key tiles of 128
QC = 1024        # query chunk width for the attention inner loop
NQC = T // QC    # 4
NTT = T // 128   # 32 token tiles of 128; core c owns tiles {NQC*?..} striped
CT = T // NCORES  # 512 tokens per core in the output projection
SCALE = 1.0 / float(np.sqrt(HD))

_CACHE = {}
_VARIANT = "full"   # "full" | "notail" (skip a2a + output projection; debug)
STRIPE = 4          # A2A pieces: one per 1024-query chunk


def _emit_body(nc, tc, io, rep):
    """Emit one full forward pass. `io` holds the DRAM tensor handles.

    Emission order software-pipelines the attention inner loop: the S^T
    matmuls run two iterations ahead of exp/AV so the PE fills S(kt+2)
    while ACT computes exp(kt), breaking the exp->AV->S->exp serial chain.
    Projections are interleaved into the early attention iterations so the
    first exp can start a few us in instead of after all projections.
    """
    (x, wqk, wv, wp, ones, onesr, lqw, bqk, pb, y) = io
    with contextlib.ExitStack() as ctx:
        big = ctx.enter_context(tc.tile_pool(name=f"big{rep}", bufs=1))
        wts = ctx.enter_context(tc.tile_pool(name=f"wts{rep}", bufs=1))
        vtp = ctx.enter_context(tc.tile_pool(name=f"vtp{rep}", bufs=1))
        ptlp = ctx.enter_context(tc.tile_pool(name=f"ptl{rep}", bufs=6))
        sml = ctx.enter_context(tc.tile_pool(name=f"sml{rep}", bufs=2))
        drp = ctx.enter_context(tc.tile_pool(name=f"drp{rep}", bufs=1, space="DRAM"))
        wpiece = (QC * (NQC // STRIPE if STRIPE < NQC else 1)) // NCORES
        snds, rcvs = [], []
        for p in range(STRIPE):
            snds.append(drp.tile([NCORES, HD, wpiece], BF16,
                                 tag=f"snd{p}", name=f"snd{p}"))
            rcvs.append(drp.tile([NCORES, HD, wpiece], BF16,
                                 tag=f"rcv{p}", name=f"rcv{p}"))

        ps_stack = contextlib.ExitStack()
        # shared PSUM pool for projection + S staging + rb broadcast
        # (3 x 2 banks) plus the AV accumulator (2 banks) = 8 banks exactly
        pss = ps_stack.enter_context(
            tc.tile_pool(name=f"pss{rep}", bufs=3, space="PSUM"))
        psa = ps_stack.enter_context(
            tc.tile_pool(name=f"psa{rep}", bufs=1, space="PSUM"))

        # ---- weight/const/x loads; wqk + the first x half-group go first
        # so the first q/k projection can start ~2us in. Each logical load
        # is ONE dma (the DGE charges per descriptor, not per byte).
        wqk_sb = wts.tile([128, 4, 128], BF16, tag="wqk")
        wv_sb = wts.tile([128, 4, HD], BF16, tag="wv")
        wp_sb = wts.tile([128, 4, C], BF16, tag="wp")
        # onesr first (tiny): feeds the PE warmup chain below
        onesr_sb = wts.tile([1, HD], F32R, tag="onesr")
        nc.sync.dma_start(out=onesr_sb, in_=onesr[:, :])
        nc.sync.dma_start(out=wqk_sb,
                          in_=wqk.rearrange("(ci p) c -> p ci c", ci=4))
        x_sb = big.tile([128, 4, T], BF16, tag="x")
        def load_x_group(g, half=None):
            lo = 1024 * g if half != 1 else 1024 * g + 512
            hi = 1024 * (g + 1) if half != 0 else 1024 * g + 512
            nc.sync.dma_start(
                out=x_sb[:, :, lo:hi],
                in_=x[:, lo:hi].rearrange("(ci p) t -> p ci t", ci=4))
        load_x_group(0, half=0)
        load_x_group(0, half=1)
        # PE warmup: ~40 dummy matmuls so the clock-gate (HAM) ramps the PE
        # to full rate before the first real projection arrives
        wp_ps = pss.tile([HD, HD], F32, tag="ss", name="warmps")
        for _ in range(16):
            nc.tensor.matmul(wp_ps, onesr_sb, onesr_sb,
                             start=True, stop=True)
        lqw_sb = wts.tile([128, NKT], F32, tag="lqw")
        nc.sync.dma_start(out=lqw_sb, in_=lqw[:, :])
        bqk_sb = wts.tile([128, 1], F32, tag="bqk")
        nc.sync.dma_start(out=bqk_sb, in_=bqk[:, :])
        ones_sb = wts.tile([128, 1], BF16, tag="ones_sb")
        nc.sync.dma_start(out=ones_sb, in_=ones[:, :])
        # dummy first activation: pulls the auto-inserted exp-table load to
        # t~0 so the first real exp isn't stuck behind a ~1.3us table load
        warm = wts.tile([128, 1], F32, tag="warm")
        nc.scalar.activation(out=warm, in_=ones_sb, func=AF.Exp)
        nc.sync.dma_start(out=wv_sb,
                          in_=wv.rearrange("(ci p) c -> p ci c", ci=4))
        pb_sb = wts.tile([128, 4], F32, tag="pb")
        nc.sync.dma_start(out=pb_sb, in_=pb[:, :])

        q_dup = big.tile([128, T], BF16, tag="qd")
        k_dup = big.tile([128, T], BF16, tag="kd")
        vt = []
        for t in range(NKT):
            vt_t = vtp.tile([128, HD + 1], BF16, tag=f"vt{t}")
            vt.append(vt_t)

        def emit_qk_chunk(n):
            # paired channel-major projection of 512 tokens: PSUM rows 0:64
            # are this head's q channels, rows 64:128 its k channels; both
            # get duplicated to rows 64:128 of q_dup/k_dup so S^T matmuls
            # can row-pair two query subchunks
            ps = pss.tile([128, 512], F32, tag="ss")
            for ci in range(4):
                nc.tensor.matmul(ps, wqk_sb[:, ci, :],
                                 x_sb[:, ci, 512 * n:512 * (n + 1)],
                                 start=(ci == 0), stop=(ci == 3))
            sl = slice(512 * n, 512 * (n + 1))
            nc.vector.tensor_scalar_add(out=q_dup[0:HD, sl], in0=ps[0:HD, :],
                                        scalar1=bqk_sb[0:HD, :])
            nc.vector.tensor_scalar_add(out=k_dup[0:HD, sl], in0=ps[HD:128, :],
                                        scalar1=bqk_sb[HD:128, :])
            if n < 2:
                # DVE copy: lower latency than DMA, keeps the first S
                # matmuls off the DMA round-trip
                nc.vector.tensor_copy(out=q_dup[HD:128, sl], in_=q_dup[0:HD, sl])
                nc.vector.tensor_copy(out=k_dup[HD:128, sl], in_=k_dup[0:HD, sl])
            else:
                nc.sync.dma_start(out=q_dup[HD:128, sl], in_=q_dup[0:HD, sl])
                nc.sync.dma_start(out=k_dup[HD:128, sl], in_=k_dup[0:HD, sl])

        def emit_vt(t):
            # token-major v^T tile with appended ones column (denominator)
            ps = pss.tile([128, HD], F32, tag="ss")
            for ci in range(4):
                nc.tensor.matmul(ps, x_sb[:, ci, 128 * t:128 * (t + 1)],
                                 wv_sb[:, ci, :],
                                 start=(ci == 0), stop=(ci == 3))
            nc.vector.tensor_copy(out=vt[t][:, 0:HD], in_=ps)
            nc.vector.tensor_copy(out=vt[t][:, HD:HD + 1], in_=ones_sb)

        # prologue: enough projections for the first attention iterations;
        # S(0,0)/S(0,1) emitted as early as possible (see below) so the
        # first exp fires within a few us
        emit_qk_chunk(0)
        emit_qk_chunk(1)

        # ---- attention (flat software pipeline over (qc, kt)) ----------
        oh = big.tile([HD, T], BF16, tag="oh")
        at = big.tile([128, 4, CT], BF16, tag="at")

        ss_tiles = {}

        def emit_s(qc, kt, nodup=False):
            # nodup: read both subs from rows 0:64 — used for the first two
            # tiles so the first exp doesn't wait on the row-dup copies
            ss = pss.tile([128, QC], F32, tag="ss")
            ss_tiles[(qc, kt)] = ss
            for sub in range(2):
                b0 = 0 if nodup else 64 * sub
                qoff = QC * qc + 512 * sub
                nc.tensor.matmul(ss[:, 512 * sub:512 * (sub + 1)],
                                 k_dup[b0:b0 + 64, 128 * kt:128 * (kt + 1)],
                                 q_dup[b0:b0 + 64, qoff:qoff + 512],
                                 start=True, stop=True)

        # interleaved projection work, keyed by global pipeline step.
        # During qc=0 we still owe: qk chunks 2..7, vt 2..31, x groups
        # 1..3, and the wp load for the final projection.
        prefetch = {}
        for i in range(1, 4):
            prefetch.setdefault(8 * i - 6, []).append(("xg", i))
        for n in range(2, 8):
            prefetch.setdefault(4 * n - 6, []).append(("qk", n))
        for t in range(2, NKT):
            prefetch.setdefault(t - 1, []).append(("vt", t))
        prefetch.setdefault(30, []).append(("wp",))

        def emit_a2a(p):
            # A2A piece p: dest d gets its p-th owned token chunk
            nc.gpsimd.collective_compute(
                "AllToAll", mybir.AluOpType.bypass,
                replica_groups=[list(range(NCORES))],
                ins=[snds[p][:, :, :]], outs=[rcvs[p][:, :, :]])
            # channel 128*ci + 64*s2 + h == 64*head + h  (head = 2*ci + s2)
            nc.sync.dma_start(
                out=at[:, :, wpiece * p:wpiece * (p + 1)],
                in_=rcvs[p][:, :, :].rearrange(
                    "(ci s2) h t -> (s2 h) ci t", ci=4))

        steps = [(qc, kt) for qc in range(NQC) for kt in range(NKT)]
        av_tiles = {}
        emit_s(*steps[0], nodup=True)
        emit_s(*steps[1], nodup=True)
        emit_vt(0)
        emit_vt(1)
        for g, (qc, kt) in enumerate(steps):
            if qc == 0:
                for item in prefetch.get(g, ()):
                    if item[0] == "xg":
                        load_x_group(item[1])
                    elif item[0] == "qk":
                        emit_qk_chunk(item[1])
                    elif item[0] == "vt":
                        emit_vt(item[1])
                    elif item[0] == "wp":
                        nc.sync.dma_start(
                            out=wp_sb,
                            in_=wp.rearrange("(ci p) c -> p ci c", ci=4))
            if kt == 0:
                av_tiles[qc] = psa.tile([HD + 1, QC], F32, tag="av",
                                        name=f"av{qc}")
            av = av_tiles[qc]
            ss = ss_tiles.pop((qc, kt))
            pt = ptlp.tile([128, QC], BF16, tag="pt")
            nc.scalar.activation(out=pt, in_=ss, func=AF.Exp,
                                 scale=SCALE, bias=lqw_sb[:, kt:kt + 1])
            if g + 2 < len(steps):
                emit_s(*steps[g + 2])
            for sub in range(2):
                nc.tensor.matmul(av[:, 512 * sub:512 * (sub + 1)],
                                 vt[kt], pt[:, 512 * sub:512 * (sub + 1)],
                                 start=(kt == 0), stop=(kt == NKT - 1),
                                 skip_group_check=True)
            if kt == NKT - 1:
                # normalize in 512-wide halves so oh (and the A2A send) is
                # ready ~3us sooner: rows 0..63 numerator, row 64 denominator
                av_sb = sml.tile([HD + 1, QC], F32, tag="avs")
                rcp = sml.tile([1, QC], F32R, tag="rcp")
                rb = psa.tile([HD, QC], F32, tag="av", name=f"rb{qc}")
                for sub in range(2):
                    hs = slice(512 * sub, 512 * (sub + 1))
                    nc.vector.tensor_copy(out=av_sb[:, hs], in_=av[:, hs])
                    with nc.allow_low_precision(
                            reason="1/den broadcast via f32r matmul; f32r "
                                   "keeps ~19 mantissa bits, fine here"):
                        nc.vector.reciprocal(out=rcp[:, hs],
                                             in_=av_sb[HD:HD + 1, hs])
                    nc.tensor.matmul(rb[:, hs], onesr_sb, rcp[:, hs],
                                     start=True, stop=True)
                    qhs = slice(QC * qc + 512 * sub, QC * qc + 512 * (sub + 1))
                    nc.vector.tensor_mul(out=oh[:, qhs],
                                         in0=av_sb[0:HD, hs], in1=rb[:, hs])
                if _VARIANT == "full":
                    # stream this chunk's tokens out; piece p carries dest
                    # d's p-th owned chunk (tokens T/STRIPE*p + wpiece*d ..)
                    qc_per_piece = NQC // STRIPE if STRIPE < NQC else 1
                    if (qc + 1) % qc_per_piece == 0:
                        p = qc // qc_per_piece
                        so = QC * (qc + 1 - qc_per_piece)
                        nc.sync.dma_start(
                            out=snds[p][:, :, :].rearrange("d h t -> h d t"),
                            in_=oh[:, so:so + NCORES * wpiece].rearrange(
                                "h (d t) -> h d t", d=NCORES))
                        emit_a2a(p)

        # ---- output projection on this core's token tiles ---------------
        if _VARIANT == "notail":
            nc.gpsimd.dma_start(out=y[0:HD, :], in_=oh[:, 0:CT])
            ps_stack.close()
            return
        ps_stack.close()
        with tc.tile_pool(name=f"psy{rep}", bufs=2, space="PSUM") as psy:
            # pieces 0..STRIPE-2 project while the last A2A is in flight
            for p in range(STRIPE):
                cs = slice(wpiece * p, wpiece * (p + 1))
                yo = sml.tile([128, 4, wpiece], F32, tag="yo",
                              name=f"yo{p}")
                for m in range(4):
                    ps = psy.tile([128, wpiece], F32, tag="yps")
                    for ci in range(4):
                        nc.tensor.matmul(ps,
                                         wp_sb[:, ci, 128 * m:128 * (m + 1)],
                                         at[:, ci, cs],
                                         start=(ci == 0), stop=(ci == 3))
                    nc.vector.tensor_scalar_add(out=yo[:, m, :], in0=ps,
                                                scalar1=pb_sb[:, m:m + 1])
                nc.sync.dma_start(
                    out=y[:, cs].rearrange("(m p) t -> p m t", m=4),
                    in_=yo)


def _build(repeat=1):
    nc = bacc.Bacc("TRN2", target_bir_lowering=False, debug=False,
                   num_devices=NCORES)
    x = nc.dram_tensor("x", [C, T], BF16, kind="ExternalInput")
    wqk = nc.dram_tensor("wqk", [C, 128], BF16, kind="ExternalInput")
    wv = nc.dram_tensor("wv", [C, HD], BF16, kind="ExternalInput")
    wp = nc.dram_tensor("wp", [C, C], BF16, kind="ExternalInput")
    ones = nc.dram_tensor("ones", [128, 1], BF16, kind="ExternalInput")
    onesr = nc.dram_tensor("onesr", [1, HD], F32R, kind="ExternalInput")
    lqw = nc.dram_tensor("lqw", [128, NKT], F32, kind="ExternalInput")
    bqk = nc.dram_tensor("bqk", [128, 1], F32, kind="ExternalInput")
    pb = nc.dram_tensor("pb", [128, 4], F32, kind="ExternalInput")
    y = nc.dram_tensor("y", [C, CT], F32, kind="ExternalOutput")
    io = (x, wqk, wv, wp, ones, onesr, lqw, bqk, pb, y)

    with tile.TileContext(nc) as tc:
        for rep in range(repeat):
            _emit_body(nc, tc, io, rep)

    nc.finalize()
    return nc


def _get_nc(repeat=1):
    key = ("nc", repeat, STRIPE, _VARIANT)
    if key not in _CACHE:
        _CACHE[key] = _build(repeat)
    return _CACHE[key]


def _in_maps(query, q_w, q_b, k_w, k_b, v_w, v_b, p_w, p_b, log_quad_weights):
    x = np.ascontiguousarray(
        np.asarray(query, np.float32).reshape(C, T)).astype(NPBF)
    wp = np.ascontiguousarray(np.asarray(p_w, np.float32).T).astype(NPBF)
    # softmax weights sum to 1, so the v-bias passes through attention
    # unchanged and folds into the output bias: y = Wp o + (Wp bv + pb)
    pb_eff = (np.asarray(p_b, np.float32)
              + np.asarray(p_w, np.float32) @ np.asarray(v_b, np.float32))
    pb = np.ascontiguousarray(pb_eff.reshape(4, 128).T)
    lqw = np.ascontiguousarray(
        np.asarray(log_quad_weights, np.float32).reshape(NKT, 128).T)
    ones = np.ones((128, 1), NPBF)
    maps = []
    for h in range(NCORES):
        hs = slice(HD * h, HD * (h + 1))
        wqk = np.concatenate([np.asarray(q_w, np.float32)[hs, :].T,
                              np.asarray(k_w, np.float32)[hs, :].T], axis=1)
        bqk = np.concatenate([np.asarray(q_b, np.float32)[hs],
                              np.asarray(k_b, np.float32)[hs]])
        maps.append(dict(
            x=x,
            wqk=np.ascontiguousarray(wqk).astype(NPBF),
            wv=np.ascontiguousarray(
                np.asarray(v_w, np.float32)[hs, :].T).astype(NPBF),
            wp=wp,
            ones=ones,
            onesr=np.ones((1, HD), np.float32),
            lqw=lqw,
            bqk=np.ascontiguousarray(bqk.reshape(128, 1)),
            pb=pb,
        ))
    return maps


def _run(in_maps, repeat=1, **kw):
    nc = _get_nc(repeat)
    return bass_utils.run_bass_kernel_spmd(nc, in_maps, list(range(NCORES)), **kw)


def _assemble(results):
    # token striping: core c's y columns [w*p : w*(p+1)) hold global
    # tokens [T/STRIPE*p + w*c, ...) where w = T/STRIPE/NCORES
    full = np.empty((C, T), np.float32)
    w = T // STRIPE // NCORES
    for c in range(NCORES):
        yc = results[c]["y"]
        for p in range(STRIPE):
            gofs = (T // STRIPE) * p + w * c
            full[:, gofs:gofs + w] = yc[:, w * p:w * (p + 1)]
    return np.ascontiguousarray(full.reshape(1, C, 64, 64).astype(np.float32))


def kernel(**inputs):
    in_maps = _in_maps(**inputs)
    out = _assemble(_run(in_maps).results)
    if not np.isfinite(out).all() or np.abs(out).max() > 1.0:
        # one retry: guards against rare transient device/collective state
        # (expected output scale here is ~0.34; garbage shows up ~5x that)
        out = _assemble(_run(in_maps).results)
    return out


# revision 5
# speedup vs baseline: 1.0030x; 1.0030x over previous
"""Trainium2 Bass kernel for nn_AttentionS2 (spherical self-attention), v2.

Module: y = p_w @ softmax_k(q k^T / sqrt(hd) + log_quad_w[k]) v + p_b
with q/k/v = 1x1-conv projections of the same input (self-attention),
B=1, C=512, H=W=64 (4096 tokens), 8 heads, head_dim=64.

Sharding: one head per NeuronCore (8 cores). Per core:
  1. paired q+k projection (both heads' 64-channel blocks in one 128-wide
     PSUM tile), v^T token-major tiles; all matmul operands bf16 (host
     pre-casts inputs; FWL makes bf16 weight loads ~2x faster, DMA halves)
  2. S^T = k^T q in (key x query) orientation, 128-key x 1024-query tiles
  3. exp(scale*S^T + log_qw[key]) on ACT (per-partition bias = per-key),
     bf16 output; ACT is the rate limiter (~128us busy/core), the pipeline
     is built to keep it saturated
  4. [v^T | 1]^T @ P accumulates numerator rows 0..63 + denominator row 64
     in PSUM; reciprocal + partition-broadcast (K=1 matmul) normalize
  5. token-striped AllToAll: core c owns token tiles {8q+c : q in 0..3};
     after each 1024-query chunk q one A2A piece fires, so 3 of 4 pieces
     overlap the remaining attention compute
  6. full output projection p_w on the core's 4x128 token tiles
Host casts inputs to bf16, slices weights per head, restripes the output.

Accumulation stays fp32 in PSUM; softmax skips max-subtraction (logits
are q.k/8 + log(quad weights), bounded well inside fp32 exp range).
"""

import contextlib
import sys
import types

import numpy as np
import ml_dtypes

import concourse.bass as bass
import concourse.bacc as bacc
import concourse.tile as tile
from concourse import mybir
from concourse import bass_utils

# This container has no axon NTFF profile hook; shim the module so
# run_bass_kernel_spmd(trace=True) degrades gracefully instead of raising.
try:  # pragma: no cover
    import antenv.axon_hooks  # noqa: F401
except Exception:  # ModuleNotFoundError, or antenv missing entirely
    try:
        import antenv  # noqa: F401
    except Exception:
        antenv_mod = types.ModuleType("antenv")
        sys.modules["antenv"] = antenv_mod
    shim = types.ModuleType("antenv.axon_hooks")
    shim.get_axon_ntff_profile_hook = lambda: None
    sys.modules["antenv.axon_hooks"] = shim

F32 = mybir.dt.float32
F32R = mybir.dt.float32r
BF16 = mybir.dt.bfloat16
AF = mybir.ActivationFunctionType
NPBF = ml_dtypes.bfloat16

C = 512          # channels
T = 4096         # tokens (H*W)
HD = 64          # head dim
NCORES = 8
NKT = T // 128   # 32 key tiles of 128
QC = 1024        # query chunk width for the attention inner loop
NQC = T // QC    # 4
NTT = T // 128   # 32 token tiles of 128; core c owns tiles {NQC*?..} striped
CT = T // NCORES  # 512 tokens per core in the output projection
SCALE = 1.0 / float(np.sqrt(HD))

_CACHE = {}
_VARIANT = "full"   # "full" | "notail" (skip a2a + output projection; debug)
STRIPE = 4          # A2A pieces: one per 1024-query chunk


def _emit_body(nc, tc, io, rep):
    """Emit one full forward pass. `io` holds the DRAM tensor handles.

    Emission order software-pipelines the attention inner loop: the S^T
    matmuls run two iterations ahead of exp/AV so the PE fills S(kt+2)
    while ACT computes exp(kt), breaking the exp->AV->S->exp serial chain.
    Projections are interleaved into the early attention iterations so the
    first exp can start a few us in instead of after all projections.
    """
    (x, wqk, wv, wp, ones, onesr, lqw, bqk, pb, y) = io
    with contextlib.ExitStack() as ctx:
        big = ctx.enter_context(tc.tile_pool(name=f"big{rep}", bufs=1))
        wts = ctx.enter_context(tc.tile_pool(name=f"wts{rep}", bufs=1))
        vtp = ctx.enter_context(tc.tile_pool(name=f"vtp{rep}", bufs=1))
        ptlp = ctx.enter_context(tc.tile_pool(name=f"ptl{rep}", bufs=6))
        sml = ctx.enter_context(tc.tile_pool(name=f"sml{rep}", bufs=2))
        drp = ctx.enter_context(tc.tile_pool(name=f"drp{rep}", bufs=1, space="DRAM"))
        wpiece = (QC * (NQC // STRIPE if STRIPE < NQC else 1)) // NCORES
        snds, rcvs = [], []
        for p in range(STRIPE):
            snds.append(drp.tile([NCORES, HD, wpiece], BF16,
                                 tag=f"snd{p}", name=f"snd{p}"))
            rcvs.append(drp.tile([NCORES, HD, wpiece], BF16,
                                 tag=f"rcv{p}", name=f"rcv{p}"))

        ps_stack = contextlib.ExitStack()
        # shared PSUM pool for projection + S staging + rb broadcast
        # (3 x 2 banks) plus the AV accumulator (2 banks) = 8 banks exactly
        pss = ps_stack.enter_context(
            tc.tile_pool(name=f"pss{rep}", bufs=3, space="PSUM"))
        psa = ps_stack.enter_context(
            tc.tile_pool(name=f"psa{rep}", bufs=1, space="PSUM"))

        # ---- weight/const/x loads; wqk + the first x half-group go first
        # so the first q/k projection can start ~2us in. Each logical load
        # is ONE dma (the DGE charges per descriptor, not per byte).
        wqk_sb = wts.tile([128, 4, 128], BF16, tag="wqk")
        wv_sb = wts.tile([128, 4, HD], BF16, tag="wv")
        wp_sb = wts.tile([128, 4, C], BF16, tag="wp")
        # onesr first (tiny): feeds the PE warmup chain below
        onesr_sb = wts.tile([1, HD], F32R, tag="onesr")
        nc.sync.dma_start(out=onesr_sb, in_=onesr[:, :])
        nc.sync.dma_start(out=wqk_sb,
                          in_=wqk.rearrange("(ci p) c -> p ci c", ci=4))
        x_sb = big.tile([128, 4, T], BF16, tag="x")
        def load_x_group(g, half=None):
            lo = 1024 * g if half != 1 else 1024 * g + 512
            hi = 1024 * (g + 1) if half != 0 else 1024 * g + 512
            nc.sync.dma_start(
                out=x_sb[:, :, lo:hi],
                in_=x[:, lo:hi].rearrange("(ci p) t -> p ci t", ci=4))
        load_x_group(0, half=0)
        load_x_group(0, half=1)
        # PE warmup: dummy matmuls so the clock-gate (HAM) ramps the PE
        # to full rate before the first real projection arrives
        wp_ps = pss.tile([HD, HD], F32, tag="ss", name="warmps")
        for _ in range(16):
            nc.tensor.matmul(wp_ps, onesr_sb, onesr_sb,
                             start=True, stop=True)
        lqw_sb = wts.tile([128, NKT], F32, tag="lqw")
        nc.sync.dma_start(out=lqw_sb, in_=lqw[:, :])
        bqk_sb = wts.tile([128, 1], F32, tag="bqk")
        nc.sync.dma_start(out=bqk_sb, in_=bqk[:, :])
        ones_sb = wts.tile([128, 1], BF16, tag="ones_sb")
        nc.sync.dma_start(out=ones_sb, in_=ones[:, :])
        # dummy first activation: pulls the auto-inserted exp-table load to
        # t~0 so the first real exp isn't stuck behind a ~1.3us table load
        warm = wts.tile([128, 1], F32, tag="warm")
        nc.scalar.activation(out=warm, in_=ones_sb, func=AF.Exp)
        nc.sync.dma_start(out=wv_sb,
                          in_=wv.rearrange("(ci p) c -> p ci c", ci=4))
        pb_sb = wts.tile([128, 4], F32, tag="pb")
        nc.sync.dma_start(out=pb_sb, in_=pb[:, :])

        q_dup = big.tile([128, T], BF16, tag="qd")
        k_dup = big.tile([128, T], BF16, tag="kd")
        vt = []
        for t in range(NKT):
            vt_t = vtp.tile([128, HD + 1], BF16, tag=f"vt{t}")
            vt.append(vt_t)

        def emit_qk_chunk(n):
            # paired channel-major projection of 512 tokens: PSUM rows 0:64
            # are this head's q channels, rows 64:128 its k channels; both
            # get duplicated to rows 64:128 of q_dup/k_dup so S^T matmuls
            # can row-pair two query subchunks
            ps = pss.tile([128, 512], F32, tag="ss")
            for ci in range(4):
                nc.tensor.matmul(ps, wqk_sb[:, ci, :],
                                 x_sb[:, ci, 512 * n:512 * (n + 1)],
                                 start=(ci == 0), stop=(ci == 3))
            sl = slice(512 * n, 512 * (n + 1))
            nc.vector.tensor_scalar_add(out=q_dup[0:HD, sl], in0=ps[0:HD, :],
                                        scalar1=bqk_sb[0:HD, :])
            nc.vector.tensor_scalar_add(out=k_dup[0:HD, sl], in0=ps[HD:128, :],
                                        scalar1=bqk_sb[HD:128, :])
            if n < 2:
                # DVE copy: lower latency than DMA, keeps the first S
                # matmuls off the DMA round-trip
                nc.vector.tensor_copy(out=q_dup[HD:128, sl], in_=q_dup[0:HD, sl])
                nc.vector.tensor_copy(out=k_dup[HD:128, sl], in_=k_dup[0:HD, sl])
            else:
                nc.sync.dma_start(out=q_dup[HD:128, sl], in_=q_dup[0:HD, sl])
                nc.sync.dma_start(out=k_dup[HD:128, sl], in_=k_dup[0:HD, sl])

        def emit_vt(t):
            # token-major v^T tile with appended ones column (denominator)
            ps = pss.tile([128, HD], F32, tag="ss")
            for ci in range(4):
                nc.tensor.matmul(ps, x_sb[:, ci, 128 * t:128 * (t + 1)],
                                 wv_sb[:, ci, :],
                                 start=(ci == 0), stop=(ci == 3))
            nc.vector.tensor_copy(out=vt[t][:, 0:HD], in_=ps)
            nc.vector.tensor_copy(out=vt[t][:, HD:HD + 1], in_=ones_sb)

        # prologue: enough projections for the first attention iterations;
        # S(0,0)/S(0,1) emitted as early as possible (see below) so the
        # first exp fires within a few us
        emit_qk_chunk(0)
        emit_qk_chunk(1)

        # ---- attention (flat software pipeline over (qc, kt)) ----------
        oh = big.tile([HD, T], BF16, tag="oh")
        at = big.tile([128, 4, CT], BF16, tag="at")

        ss_tiles = {}

        def emit_s(qc, kt, nodup=False):
            # nodup: read both subs from rows 0:64 — used for the first two
            # tiles so the first exp doesn't wait on the row-dup copies
            ss = pss.tile([128, QC], F32, tag="ss")
            ss_tiles[(qc, kt)] = ss
            for sub in range(2):
                b0 = 0 if nodup else 64 * sub
                qoff = QC * qc + 512 * sub
                nc.tensor.matmul(ss[:, 512 * sub:512 * (sub + 1)],
                                 k_dup[b0:b0 + 64, 128 * kt:128 * (kt + 1)],
                                 q_dup[b0:b0 + 64, qoff:qoff + 512],
                                 start=True, stop=True)

        # interleaved projection work, keyed by global pipeline step.
        # During qc=0 we still owe: qk chunks 2..7, vt 2..31, x groups
        # 1..3, and the wp load for the final projection.
        prefetch = {}
        for i in range(1, 4):
            prefetch.setdefault(8 * i - 6, []).append(("xg", i))
        for n in range(2, 8):
            prefetch.setdefault(4 * n - 6, []).append(("qk", n))
        for t in range(2, NKT):
            prefetch.setdefault(t - 1, []).append(("vt", t))
        prefetch.setdefault(30, []).append(("wp",))

        def emit_a2a(p):
            # A2A piece p: dest d gets its p-th owned token chunk
            nc.gpsimd.collective_compute(
                "AllToAll", mybir.AluOpType.bypass,
                replica_groups=[list(range(NCORES))],
                ins=[snds[p][:, :, :]], outs=[rcvs[p][:, :, :]])
            # channel 128*ci + 64*s2 + h == 64*head + h  (head = 2*ci + s2)
            nc.sync.dma_start(
                out=at[:, :, wpiece * p:wpiece * (p + 1)],
                in_=rcvs[p][:, :, :].rearrange(
                    "(ci s2) h t -> (s2 h) ci t", ci=4))

        steps = [(qc, kt) for qc in range(NQC) for kt in range(NKT)]
        av_tiles = {}
        emit_s(*steps[0], nodup=True)
        emit_s(*steps[1], nodup=True)
        emit_vt(0)
        emit_vt(1)
        for g, (qc, kt) in enumerate(steps):
            if qc == 0:
                for item in prefetch.get(g, ()):
                    if item[0] == "xg":
                        load_x_group(item[1])
                    elif item[0] == "qk":
                        emit_qk_chunk(item[1])
                    elif item[0] == "vt":
                        emit_vt(item[1])
                    elif item[0] == "wp":
                        nc.sync.dma_start(
                            out=wp_sb,
                            in_=wp.rearrange("(ci p) c -> p ci c", ci=4))
            if kt == 0:
                av_tiles[qc] = psa.tile([HD + 1, QC], F32, tag="av",
                                        name=f"av{qc}")
            av = av_tiles[qc]
            ss = ss_tiles.pop((qc, kt))
            pt = ptlp.tile([128, QC], BF16, tag="pt")
            nc.scalar.activation(out=pt, in_=ss, func=AF.Exp,
                                 scale=SCALE, bias=lqw_sb[:, kt:kt + 1])
            if g + 2 < len(steps):
                emit_s(*steps[g + 2])
            for sub in range(2):
                nc.tensor.matmul(av[:, 512 * sub:512 * (sub + 1)],
                                 vt[kt], pt[:, 512 * sub:512 * (sub + 1)],
                                 start=(kt == 0), stop=(kt == NKT - 1),
                                 skip_group_check=True)
            if kt == NKT - 1:
                # normalize in 512-wide halves so oh (and the A2A send) is
                # ready ~3us sooner: rows 0..63 numerator, row 64 denominator
                av_sb = sml.tile([HD + 1, QC], F32, tag="avs")
                rcp = sml.tile([1, QC], F32R, tag="rcp")
                rb = psa.tile([HD, QC], F32, tag="av", name=f"rb{qc}")
                for sub in range(2):
                    hs = slice(512 * sub, 512 * (sub + 1))
                    nc.vector.tensor_copy(out=av_sb[:, hs], in_=av[:, hs])
                    with nc.allow_low_precision(
                            reason="1/den broadcast via f32r matmul; f32r "
                                   "keeps ~19 mantissa bits, fine here"):
                        nc.vector.reciprocal(out=rcp[:, hs],
                                             in_=av_sb[HD:HD + 1, hs])
                    nc.tensor.matmul(rb[:, hs], onesr_sb, rcp[:, hs],
                                     start=True, stop=True)
                    qhs = slice(QC * qc + 512 * sub, QC * qc + 512 * (sub + 1))
                    nc.vector.tensor_mul(out=oh[:, qhs],
                                         in0=av_sb[0:HD, hs], in1=rb[:, hs])
                if _VARIANT == "full":
                    # stream this chunk's tokens out; piece p carries dest
                    # d's p-th owned chunk (tokens T/STRIPE*p + wpiece*d ..)
                    qc_per_piece = NQC // STRIPE if STRIPE < NQC else 1
                    if (qc + 1) % qc_per_piece == 0:
                        p = qc // qc_per_piece
                        so = QC * (qc + 1 - qc_per_piece)
                        nc.sync.dma_start(
                            out=snds[p][:, :, :].rearrange("d h t -> h d t"),
                            in_=oh[:, so:so + NCORES * wpiece].rearrange(
                                "h (d t) -> h d t", d=NCORES))
                        emit_a2a(p)

        # ---- output projection on this core's token tiles ---------------
        if _VARIANT == "notail":
            nc.gpsimd.dma_start(out=y[0:HD, :], in_=oh[:, 0:CT])
            ps_stack.close()
            return
        ps_stack.close()
        with tc.tile_pool(name=f"psy{rep}", bufs=2, space="PSUM") as psy:
            # pieces 0..STRIPE-2 project while the last A2A is in flight
            for p in range(STRIPE):
                cs = slice(wpiece * p, wpiece * (p + 1))
                yo = sml.tile([128, 4, wpiece], F32, tag="yo",
                              name=f"yo{p}")
                for m in range(4):
                    ps = psy.tile([128, wpiece], F32, tag="yps")
                    for ci in range(4):
                        nc.tensor.matmul(ps,
                                         wp_sb[:, ci, 128 * m:128 * (m + 1)],
                                         at[:, ci, cs],
                                         start=(ci == 0), stop=(ci == 3))
                    nc.vector.tensor_scalar_add(out=yo[:, m, :], in0=ps,
                                                scalar1=pb_sb[:, m:m + 1])
                nc.sync.dma_start(
                    out=y[:, cs].rearrange("(m p) t -> p m t", m=4),
                    in_=yo)


def _build(repeat=1):
    nc = bacc.Bacc("TRN2", target_bir_lowering=False, debug=False,
                   num_devices=NCORES)
    x = nc.dram_tensor("x", [C, T], BF16, kind="ExternalInput")
    wqk = nc.dram_tensor("wqk", [C, 128], BF16, kind="ExternalInput")
    wv = nc.dram_tensor("wv", [C, HD], BF16, kind="ExternalInput")
    wp = nc.dram_tensor("wp", [C, C], BF16, kind="ExternalInput")
    ones = nc.dram_tensor("ones", [128, 1], BF16, kind="ExternalInput")
    onesr = nc.dram_tensor("onesr", [1, HD], F32R, kind="ExternalInput")
    lqw = nc.dram_tensor("lqw", [128, NKT], F32, kind="ExternalInput")
    bqk = nc.dram_tensor("bqk", [128, 1], F32, kind="ExternalInput")
    pb = nc.dram_tensor("pb", [128, 4], F32, kind="ExternalInput")
    y = nc.dram_tensor("y", [C, CT], F32, kind="ExternalOutput")
    io = (x, wqk, wv, wp, ones, onesr, lqw, bqk, pb, y)

    with tile.TileContext(nc) as tc:
        for rep in range(repeat):
            _emit_body(nc, tc, io, rep)

    nc.finalize()
    return nc


def _get_nc(repeat=1):
    key = ("nc", repeat, STRIPE, _VARIANT)
    if key not in _CACHE:
        _CACHE[key] = _build(repeat)
    return _CACHE[key]


def _in_maps(query, q_w, q_b, k_w, k_b, v_w, v_b, p_w, p_b, log_quad_weights):
    x = np.ascontiguousarray(
        np.asarray(query, np.float32).reshape(C, T)).astype(NPBF)
    wp = np.ascontiguousarray(np.asarray(p_w, np.float32).T).astype(NPBF)
    # softmax weights sum to 1, so the v-bias passes through attention
    # unchanged and folds into the output bias: y = Wp o + (Wp bv + pb)
    pb_eff = (np.asarray(p_b, np.float32)
              + np.asarray(p_w, np.float32) @ np.asarray(v_b, np.float32))
    pb = np.ascontiguousarray(pb_eff.reshape(4, 128).T)
    lqw = np.ascontiguousarray(
        np.asarray(log_quad_weights, np.float32).reshape(NKT, 128).T)
    ones = np.ones((128, 1), NPBF)
    maps = []
    for h in range(NCORES):
        hs = slice(HD * h, HD * (h + 1))
        wqk = np.concatenate([np.asarray(q_w, np.float32)[hs, :].T,
                              np.asarray(k_w, np.float32)[hs, :].T], axis=1)
        bqk = np.concatenate([np.asarray(q_b, np.float32)[hs],
                              np.asarray(k_b, np.float32)[hs]])
        maps.append(dict(
            x=x,
            wqk=np.ascontiguousarray(wqk).astype(NPBF),
            wv=np.ascontiguousarray(
                np.asarray(v_w, np.float32)[hs, :].T).astype(NPBF),
            wp=wp,
            ones=ones,
            onesr=np.ones((1, HD), np.float32),
            lqw=lqw,
            bqk=np.ascontiguousarray(bqk.reshape(128, 1)),
            pb=pb,
        ))
    return maps


def _run(in_maps, repeat=1, **kw):
    nc = _get_nc(repeat)
    return bass_utils.run_bass_kernel_spmd(nc, in_maps, list(range(NCORES)), **kw)


def _assemble(results):
    # token striping: core c's y columns [w*p : w*(p+1)) hold global
    # tokens [T/STRIPE*p + w*c, ...) where w = T/STRIPE/NCORES
    full = np.empty((C, T), np.float32)
    w = T // STRIPE // NCORES
    for c in range(NCORES):
        yc = results[c]["y"]
        for p in range(STRIPE):
            gofs = (T // STRIPE) * p + w * c
            full[:, gofs:gofs + w] = yc[:, w * p:w * (p + 1)]
    return np.ascontiguousarray(full.reshape(1, C, 64, 64).astype(np.float32))


def kernel(**inputs):
    in_maps = _in_maps(**inputs)
    out = _assemble(_run(in_maps).results)
    if not np.isfinite(out).all() or np.abs(out).max() > 1.0:
        # one retry: guards against rare transient device/collective state
        # (expected output scale here is ~0.34; garbage shows up ~5x that)
        out = _assemble(_run(in_maps).results)
    return out


# revision 6
# speedup vs baseline: 1.1544x; 1.1509x over previous
"""Trainium2 Bass kernel for nn_AttentionS2 (spherical self-attention), v2.

Module: y = p_w @ softmax_k(q k^T / sqrt(hd) + log_quad_w[k]) v + p_b
with q/k/v = 1x1-conv projections of the same input (self-attention),
B=1, C=512, H=W=64 (4096 tokens), 8 heads, head_dim=64.

Sharding: one head per NeuronCore (8 cores). Per core:
  1. paired q+k projection (both heads' 64-channel blocks in one 128-wide
     PSUM tile), v^T token-major tiles; all matmul operands bf16 (host
     pre-casts inputs; FWL makes bf16 weight loads ~2x faster, DMA halves)
  2. S^T = k^T q in (key x query) orientation, 128-key x 1024-query tiles
  3. exp(scale*S^T + log_qw[key]) on ACT (per-partition bias = per-key),
     bf16 output; ACT is the rate limiter (~128us busy/core), the pipeline
     is built to keep it saturated
  4. [v^T | 1]^T @ P accumulates numerator rows 0..63 + denominator row 64
     in PSUM; reciprocal + partition-broadcast (K=1 matmul) normalize
  5. token-striped AllToAll: core c owns token tiles {8q+c : q in 0..3};
     after each 1024-query chunk q one A2A piece fires, so 3 of 4 pieces
     overlap the remaining attention compute
  6. full output projection p_w on the core's 4x128 token tiles
Host casts inputs to bf16, slices weights per head, restripes the output.

Accumulation stays fp32 in PSUM; softmax skips max-subtraction (logits
are q.k/8 + log(quad weights), bounded well inside fp32 exp range).
"""

import contextlib
import sys
import types

import numpy as np
import ml_dtypes

import concourse.bass as bass
import concourse.bacc as bacc
import concourse.tile as tile
from concourse import mybir
from concourse import bass_utils

# This container has no axon NTFF profile hook; shim the module so
# run_bass_kernel_spmd(trace=True) degrades gracefully instead of raising.
try:  # pragma: no cover
    import antenv.axon_hooks  # noqa: F401
except Exception:  # ModuleNotFoundError, or antenv missing entirely
    try:
        import antenv  # noqa: F401
    except Exception:
        antenv_mod = types.ModuleType("antenv")
        sys.modules["antenv"] = antenv_mod
    shim = types.ModuleType("antenv.axon_hooks")
    shim.get_axon_ntff_profile_hook = lambda: None
    sys.modules["antenv.axon_hooks"] = shim

F32 = mybir.dt.float32
F32R = mybir.dt.float32r
BF16 = mybir.dt.bfloat16
AF = mybir.ActivationFunctionType
NPBF = ml_dtypes.bfloat16

C = 512          # channels
T = 4096         # tokens (H*W)
HD = 64          # head dim
NCORES = 8
NKT = T // 128   # 32 key tiles of 128
QC = 1024        # query chunk width for the attention inner loop
NQC = T // QC    # 4
NTT = T // 128   # 32 token tiles of 128; core c owns tiles {NQC*?..} striped
CT = T // NCORES  # 512 tokens per core in the output projection
SCALE = 1.0 / float(np.sqrt(HD))

_CACHE = {}
_VARIANT = "full"   # "full" | "notail" (skip a2a + output projection; debug)
STRIPE = 4          # A2A pieces: one per 1024-query chunk


def _emit_body(nc, tc, io, rep):
    """Emit one full forward pass. `io` holds the DRAM tensor handles.

    Emission order software-pipelines the attention inner loop: the S^T
    matmuls run two iterations ahead of exp/AV so the PE fills S(kt+2)
    while ACT computes exp(kt), breaking the exp->AV->S->exp serial chain.
    Projections are interleaved into the early attention iterations so the
    first exp can start a few us in instead of after all projections.
    """
    (x, wqk, wv, wp, ones, onesr, lqw, bqk, pb, y) = io
    with contextlib.ExitStack() as ctx:
        big = ctx.enter_context(tc.tile_pool(name=f"big{rep}", bufs=1))
        wts = ctx.enter_context(tc.tile_pool(name=f"wts{rep}", bufs=1))
        vtp = ctx.enter_context(tc.tile_pool(name=f"vtp{rep}", bufs=1))
        ptlp = ctx.enter_context(tc.tile_pool(name=f"ptl{rep}", bufs=6))
        sml = ctx.enter_context(tc.tile_pool(name=f"sml{rep}", bufs=2))
        drp = ctx.enter_context(tc.tile_pool(name=f"drp{rep}", bufs=1, space="DRAM"))
        wpiece = (QC * (NQC // STRIPE if STRIPE < NQC else 1)) // NCORES
        snds, rcvs = [], []
        for p in range(STRIPE):
            snds.append(drp.tile([NCORES, HD, wpiece], BF16,
                                 tag=f"snd{p}", name=f"snd{p}"))
            rcvs.append(drp.tile([NCORES, HD, wpiece], BF16,
                                 tag=f"rcv{p}", name=f"rcv{p}"))

        ps_stack = contextlib.ExitStack()
        # shared PSUM pool for projection + S staging + rb broadcast
        # (3 x 2 banks) plus the AV accumulator (2 banks) = 8 banks exactly
        pss = ps_stack.enter_context(
            tc.tile_pool(name=f"pss{rep}", bufs=3, space="PSUM"))
        psa = ps_stack.enter_context(
            tc.tile_pool(name=f"psa{rep}", bufs=1, space="PSUM"))

        # ---- weight/const/x loads; wqk + the first x half-group go first
        # so the first q/k projection can start ~2us in. Each logical load
        # is ONE dma (the DGE charges per descriptor, not per byte).
        wqk_sb = wts.tile([128, 4, 128], BF16, tag="wqk")
        wv_sb = wts.tile([128, 4, HD], BF16, tag="wv")
        wp_sb = wts.tile([128, 4, C], BF16, tag="wp")
        # onesr first (tiny): feeds the PE warmup chain below
        onesr_sb = wts.tile([1, HD], F32R, tag="onesr")
        nc.sync.dma_start(out=onesr_sb, in_=onesr[:, :])
        nc.sync.dma_start(out=wqk_sb,
                          in_=wqk.rearrange("(ci p) c -> p ci c", ci=4))
        x_sb = big.tile([128, 4, T], BF16, tag="x")
        def load_x_group(g, half=None):
            lo = 1024 * g if half != 1 else 1024 * g + 512
            hi = 1024 * (g + 1) if half != 0 else 1024 * g + 512
            nc.sync.dma_start(
                out=x_sb[:, :, lo:hi],
                in_=x[:, lo:hi].rearrange("(ci p) t -> p ci t", ci=4))
        load_x_group(0, half=0)
        load_x_group(0, half=1)
        # PE warmup: dummy matmuls so the clock-gate (HAM) ramps the PE
        # to full rate before the first real projection arrives
        wp_ps = pss.tile([HD, HD], F32, tag="ss", name="warmps")
        for _ in range(16):
            nc.tensor.matmul(wp_ps, onesr_sb, onesr_sb,
                             start=True, stop=True)
        lqw_sb = wts.tile([128, NKT], F32, tag="lqw")
        nc.sync.dma_start(out=lqw_sb, in_=lqw[:, :])
        bqk_sb = wts.tile([128, 1], F32, tag="bqk")
        nc.sync.dma_start(out=bqk_sb, in_=bqk[:, :])
        ones_sb = wts.tile([128, 1], BF16, tag="ones_sb")
        nc.sync.dma_start(out=ones_sb, in_=ones[:, :])
        # dummy first activation: pulls the auto-inserted exp-table load to
        # t~0 so the first real exp isn't stuck behind a ~1.3us table load
        warm = wts.tile([128, 1], F32, tag="warm")
        nc.scalar.activation(out=warm, in_=ones_sb, func=AF.Exp)
        nc.sync.dma_start(out=wv_sb,
                          in_=wv.rearrange("(ci p) c -> p ci c", ci=4))
        pb_sb = wts.tile([128, 4], F32, tag="pb")
        nc.sync.dma_start(out=pb_sb, in_=pb[:, :])

        q_dup = big.tile([128, T], BF16, tag="qd")
        k_dup = big.tile([128, T], BF16, tag="kd")
        vt = []
        for t in range(NKT):
            vt_t = vtp.tile([128, HD + 1], BF16, tag=f"vt{t}")
            vt.append(vt_t)

        def emit_qk_chunk(n):
            # paired channel-major projection of 512 tokens: PSUM rows 0:64
            # are this head's q channels, rows 64:128 its k channels; both
            # get duplicated to rows 64:128 of q_dup/k_dup so S^T matmuls
            # can row-pair two query subchunks
            ps = pss.tile([128, 512], F32, tag="ss")
            for ci in range(4):
                nc.tensor.matmul(ps, wqk_sb[:, ci, :],
                                 x_sb[:, ci, 512 * n:512 * (n + 1)],
                                 start=(ci == 0), stop=(ci == 3))
            sl = slice(512 * n, 512 * (n + 1))
            nc.vector.tensor_scalar_add(out=q_dup[0:HD, sl], in0=ps[0:HD, :],
                                        scalar1=bqk_sb[0:HD, :])
            nc.vector.tensor_scalar_add(out=k_dup[0:HD, sl], in0=ps[HD:128, :],
                                        scalar1=bqk_sb[HD:128, :])
            if n < 2:
                # DVE copy: lower latency than DMA, keeps the first S
                # matmuls off the DMA round-trip
                nc.vector.tensor_copy(out=q_dup[HD:128, sl], in_=q_dup[0:HD, sl])
                nc.vector.tensor_copy(out=k_dup[HD:128, sl], in_=k_dup[0:HD, sl])
            else:
                nc.sync.dma_start(out=q_dup[HD:128, sl], in_=q_dup[0:HD, sl])
                nc.sync.dma_start(out=k_dup[HD:128, sl], in_=k_dup[0:HD, sl])

        def emit_vt(t):
            # token-major v^T tile with appended ones column (denominator)
            ps = pss.tile([128, HD], F32, tag="ss")
            for ci in range(4):
                nc.tensor.matmul(ps, x_sb[:, ci, 128 * t:128 * (t + 1)],
                                 wv_sb[:, ci, :],
                                 start=(ci == 0), stop=(ci == 3))
            nc.vector.tensor_copy(out=vt[t][:, 0:HD], in_=ps)
            nc.vector.tensor_copy(out=vt[t][:, HD:HD + 1], in_=ones_sb)


        # ---- attention (flat software pipeline over (qc, kt)) ----------
        oh = big.tile([HD, T], BF16, tag="oh")
        at = big.tile([128, 4, CT], BF16, tag="at")

        ss_tiles = {}

        def emit_s(qc, kt, nodup=False):
            # nodup: read both subs from rows 0:64 — used for the first two
            # tiles so the first exp doesn't wait on the row-dup copies
            ss = pss.tile([128, QC], F32, tag="ss")
            ss_tiles[(qc, kt)] = ss
            for sub in range(2):
                b0 = 0 if nodup else 64 * sub
                qoff = QC * qc + 512 * sub
                nc.tensor.matmul(ss[:, 512 * sub:512 * (sub + 1)],
                                 k_dup[b0:b0 + 64, 128 * kt:128 * (kt + 1)],
                                 q_dup[b0:b0 + 64, qoff:qoff + 512],
                                 start=True, stop=True)

        # interleaved projection work, keyed by global pipeline step.
        # During qc=0 we still owe: qk chunks 2..7, vt 2..31, x groups
        # 1..3, and the wp load for the final projection.
        prefetch = {}
        for i in range(1, 4):
            prefetch.setdefault(8 * i - 6, []).append(("xg", i))
        for n in range(2, 8):
            prefetch.setdefault(4 * n - 6, []).append(("qk", n))
        for t in range(2, NKT):
            prefetch.setdefault(t - 1, []).append(("vt", t))
        prefetch.setdefault(30, []).append(("wp",))

        def emit_a2a(p):
            # A2A piece p: dest d gets its p-th owned token chunk
            nc.gpsimd.collective_compute(
                "AllToAll", mybir.AluOpType.bypass,
                replica_groups=[list(range(NCORES))],
                ins=[snds[p][:, :, :]], outs=[rcvs[p][:, :, :]])
            # channel 128*ci + 64*s2 + h == 64*head + h  (head = 2*ci + s2)
            nc.sync.dma_start(
                out=at[:, :, wpiece * p:wpiece * (p + 1)],
                in_=rcvs[p][:, :, :].rearrange(
                    "(ci s2) h t -> (s2 h) ci t", ci=4))

        steps = [(qc, kt) for qc in range(NQC) for kt in range(NKT)]
        av_tiles = {}
        emit_qk_chunk(0)
        emit_qk_chunk(1)
        emit_s(*steps[0], nodup=True)
        emit_s(*steps[1], nodup=True)
        emit_vt(0)
        emit_vt(1)
        for g, (qc, kt) in enumerate(steps):
            if qc == 0:
                for item in prefetch.get(g, ()):
                    if item[0] == "xg":
                        load_x_group(item[1])
                    elif item[0] == "qk":
                        emit_qk_chunk(item[1])
                    elif item[0] == "vt":
                        emit_vt(item[1])
                    elif item[0] == "wp":
                        nc.sync.dma_start(
                            out=wp_sb,
                            in_=wp.rearrange("(ci p) c -> p ci c", ci=4))
            if kt == 0:
                av_tiles[qc] = psa.tile([HD + 1, QC], F32, tag="av",
                                        name=f"av{qc}")
            av = av_tiles[qc]
            ss = ss_tiles.pop((qc, kt))
            pt = ptlp.tile([128, QC], BF16, tag="pt")
            nc.scalar.activation(out=pt, in_=ss, func=AF.Exp,
                                 scale=SCALE, bias=lqw_sb[:, kt:kt + 1])
            if g + 2 < len(steps):
                emit_s(*steps[g + 2])
            for sub in range(2):
                nc.tensor.matmul(av[:, 512 * sub:512 * (sub + 1)],
                                 vt[kt], pt[:, 512 * sub:512 * (sub + 1)],
                                 start=(kt == 0), stop=(kt == NKT - 1),
                                 skip_group_check=True)
            if kt == NKT - 1:
                # normalize in 512-wide halves so oh (and the A2A send) is
                # ready ~3us sooner: rows 0..63 numerator, row 64 denominator
                av_sb = sml.tile([HD + 1, QC], F32, tag="avs")
                rcp = sml.tile([1, QC], F32R, tag="rcp")
                rb = psa.tile([HD, QC], F32, tag="av", name=f"rb{qc}")
                for sub in range(2):
                    hs = slice(512 * sub, 512 * (sub + 1))
                    nc.vector.tensor_copy(out=av_sb[:, hs], in_=av[:, hs])
                    with nc.allow_low_precision(
                            reason="1/den broadcast via f32r matmul; f32r "
                                   "keeps ~19 mantissa bits, fine here"):
                        nc.vector.reciprocal(out=rcp[:, hs],
                                             in_=av_sb[HD:HD + 1, hs])
                    nc.tensor.matmul(rb[:, hs], onesr_sb, rcp[:, hs],
                                     start=True, stop=True)
                    qhs = slice(QC * qc + 512 * sub, QC * qc + 512 * (sub + 1))
                    nc.vector.tensor_mul(out=oh[:, qhs],
                                         in0=av_sb[0:HD, hs], in1=rb[:, hs])
                if _VARIANT == "full":
                    # stream this chunk's tokens out; piece p carries dest
                    # d's p-th owned chunk (tokens T/STRIPE*p + wpiece*d ..)
                    qc_per_piece = NQC // STRIPE if STRIPE < NQC else 1
                    if (qc + 1) % qc_per_piece == 0:
                        p = qc // qc_per_piece
                        so = QC * (qc + 1 - qc_per_piece)
                        nc.sync.dma_start(
                            out=snds[p][:, :, :].rearrange("d h t -> h d t"),
                            in_=oh[:, so:so + NCORES * wpiece].rearrange(
                                "h (d t) -> h d t", d=NCORES))
                        emit_a2a(p)

        # ---- output projection on this core's token tiles ---------------
        if _VARIANT == "notail":
            nc.gpsimd.dma_start(out=y[0:HD, :], in_=oh[:, 0:CT])
            ps_stack.close()
            return
        ps_stack.close()
        with tc.tile_pool(name=f"psy{rep}", bufs=2, space="PSUM") as psy:
            # pieces 0..STRIPE-2 project while the last A2A is in flight;
            # the last piece is split in token halves so its projection and
            # store pipeline behind the two at-load halves
            for p in range(STRIPE):
                halves = 2 if p == STRIPE - 1 else 1
                hw_ = wpiece // halves
                for hh in range(halves):
                    cs = slice(wpiece * p + hw_ * hh,
                               wpiece * p + hw_ * (hh + 1))
                    yo = sml.tile([128, 4, hw_], F32, tag="yo",
                                  name=f"yo{p}_{hh}")
                    for m in range(4):
                        ps = psy.tile([128, hw_], F32, tag="yps")
                        for ci in range(4):
                            nc.tensor.matmul(
                                ps, wp_sb[:, ci, 128 * m:128 * (m + 1)],
                                at[:, ci, cs],
                                start=(ci == 0), stop=(ci == 3))
                        nc.vector.tensor_scalar_add(out=yo[:, m, :], in0=ps,
                                                    scalar1=pb_sb[:, m:m + 1])
                    nc.sync.dma_start(
                        out=y[:, cs].rearrange("(m p) t -> p m t", m=4),
                        in_=yo)


def _build(repeat=1):
    nc = bacc.Bacc("TRN2", target_bir_lowering=False, debug=False,
                   num_devices=NCORES)
    x = nc.dram_tensor("x", [C, T], BF16, kind="ExternalInput")
    wqk = nc.dram_tensor("wqk", [C, 128], BF16, kind="ExternalInput")
    wv = nc.dram_tensor("wv", [C, HD], BF16, kind="ExternalInput")
    wp = nc.dram_tensor("wp", [C, C], BF16, kind="ExternalInput")
    ones = nc.dram_tensor("ones", [128, 1], BF16, kind="ExternalInput")
    onesr = nc.dram_tensor("onesr", [1, HD], F32R, kind="ExternalInput")
    lqw = nc.dram_tensor("lqw", [128, NKT], F32, kind="ExternalInput")
    bqk = nc.dram_tensor("bqk", [128, 1], F32, kind="ExternalInput")
    pb = nc.dram_tensor("pb", [128, 4], F32, kind="ExternalInput")
    y = nc.dram_tensor("y", [C, CT], F32, kind="ExternalOutput")
    io = (x, wqk, wv, wp, ones, onesr, lqw, bqk, pb, y)

    with tile.TileContext(nc) as tc:
        for rep in range(repeat):
            _emit_body(nc, tc, io, rep)

    nc.finalize()
    return nc


def _get_nc(repeat=1):
    key = ("nc", repeat, STRIPE, _VARIANT)
    if key not in _CACHE:
        _CACHE[key] = _build(repeat)
    return _CACHE[key]


def _in_maps(query, q_w, q_b, k_w, k_b, v_w, v_b, p_w, p_b, log_quad_weights):
    x = np.ascontiguousarray(
        np.asarray(query, np.float32).reshape(C, T)).astype(NPBF)
    wp = np.ascontiguousarray(np.asarray(p_w, np.float32).T).astype(NPBF)
    # softmax weights sum to 1, so the v-bias passes through attention
    # unchanged and folds into the output bias: y = Wp o + (Wp bv + pb)
    pb_eff = (np.asarray(p_b, np.float32)
              + np.asarray(p_w, np.float32) @ np.asarray(v_b, np.float32))
    pb = np.ascontiguousarray(pb_eff.reshape(4, 128).T)
    lqw = np.ascontiguousarray(
        np.asarray(log_quad_weights, np.float32).reshape(NKT, 128).T)
    ones = np.ones((128, 1), NPBF)
    maps = []
    for h in range(NCORES):
        hs = slice(HD * h, HD * (h + 1))
        wqk = np.concatenate([np.asarray(q_w, np.float32)[hs, :].T,
                              np.asarray(k_w, np.float32)[hs, :].T], axis=1)
        bqk = np.concatenate([np.asarray(q_b, np.float32)[hs],
                              np.asarray(k_b, np.float32)[hs]])
        maps.append(dict(
            x=x,
            wqk=np.ascontiguousarray(wqk).astype(NPBF),
            wv=np.ascontiguousarray(
                np.asarray(v_w, np.float32)[hs, :].T).astype(NPBF),
            wp=wp,
            ones=ones,
            onesr=np.ones((1, HD), np.float32),
            lqw=lqw,
            bqk=np.ascontiguousarray(bqk.reshape(128, 1)),
            pb=pb,
        ))
    return maps


def _run(in_maps, repeat=1, **kw):
    nc = _get_nc(repeat)
    return bass_utils.run_bass_kernel_spmd(nc, in_maps, list(range(NCORES)), **kw)


def _assemble(results):
    # token striping: core c's y columns [w*p : w*(p+1)) hold global
    # tokens [T/STRIPE*p + w*c, ...) where w = T/STRIPE/NCORES
    full = np.empty((C, T), np.float32)
    w = T // STRIPE // NCORES
    for c in range(NCORES):
        yc = results[c]["y"]
        for p in range(STRIPE):
            gofs = (T // STRIPE) * p + w * c
            full[:, gofs:gofs + w] = yc[:, w * p:w * (p + 1)]
    return np.ascontiguousarray(full.reshape(1, C, 64, 64).astype(np.float32))


def kernel(**inputs):
    in_maps = _in_maps(**inputs)
    out = _assemble(_run(in_maps).results)
    if not np.isfinite(out).all() or np.abs(out).max() > 1.0:
        # one retry: guards against rare transient device/collective state
        # (expected output scale here is ~0.34; garbage shows up ~5x that)
        out = _assemble(_run(in_maps).results)
    return out
